# revision 8
# baseline (speedup 1.0000x reference)
"""Trainium2 Bass kernel for nn_Decoder (LSTM slot decoder, ragged sequences).

Sharding: data-parallel over batch across 8 NeuronCores (8 examples/core),
small weights replicated (spec sharding_hint). Per core the LSTM scan runs
sequentially over T=1024 steps with gates held in a [128-partition
gate-chunk, batch] layout so the elementwise cell runs at full vector
width. Matmuls are bf16 (measured ~1.7e-3 end-to-end rel err vs the fp32
reference; the LSTM contracts argmax-flip perturbations); state, PSUM
accumulation and activations stay fp32.

Per-step (B=8 local, H=512, S=128, 4H=2048 gates):
  gates = W_h h + P'' onehot_prev + g_enc[t] + g_const     (PSUM accum)
  c = sig(f) c + sig(i) tanh(g);  h = sig(o) tanh(c)
  slot = h W_slot^T (+b);  logp = log_softmax(slot) * valid[t] -> HBM
  onehot = (slot == rowmax)    # emb lookup becomes a matmul with P''
with W_h = W_ih[:, :H] + W_hh (both multiply the carry h) and
P'' = slot_emb_table @ W_emb^T. g_enc[t] = W_enc enc_t + g_const is
precomputed in 32-step double-buffered blocks interleaved with the scan.
The state freeze past seq_len in the reference is unnecessary: frozen
steps only produce outputs that are masked to zero, so we skip it.
"""

import numpy as np
import ml_dtypes

import concourse.bass as bass
import concourse.mybir as mybir
import concourse.tile as tile
from concourse import bacc
from concourse.bass import ds
from concourse.bass_utils import run_bass_kernel_spmd
from concourse.masks import make_identity

dt = mybir.dt
AF = mybir.ActivationFunctionType
ALU = mybir.AluOpType
AXX = mybir.AxisListType.X
P = 128
B, T, H, E, S, I = 64, 1024, 512, 128, 128, 32
NC = 8
BL = B // NC          # local batch
BLK = 32              # precompute block size (steps)
TPAD = T + 2 * BLK
bf16 = ml_dtypes.bfloat16

_cache = {}


def build_kernel():
    nc = bacc.Bacc(None, target_bir_lowering=False)
    di = lambda n, sh, d=dt.bfloat16: nc.dram_tensor(n, sh, d, kind="ExternalInput")
    encT = di("encT", [H, BL, TPAD])
    encN = di("encN", [BL, T, H])
    whT = di("whT", [H, 4 * H])
    pT = di("pT", [S, 4 * H])
    wencT = di("wencT", [H, 4 * H])
    wctxT = di("wctxT", [H, 4 * H])
    wsT = di("wsT", [H, S])
    wintT = di("wintT", [2 * H, I])
    wembT = di("wembT", [E, 4 * H])
    initT8 = di("initT8", [E, BL])
    attw = di("attw", [P, 4])
    bias_g = di("bias_g", [P, 16], dt.float32)
    b_slot = di("b_slot", [BL, S], dt.float32)
    b_int = di("b_int", [BL, I], dt.float32)
    lens = di("lens", [BL, 1], dt.float32)
    h0 = di("h0", [P, 4 * BL], dt.float32)
    c0 = di("c0", [P, 4 * BL], dt.float32)
    out_lp = nc.dram_tensor("out_lp", [BL, T, S], dt.float32, kind="ExternalOutput")
    out_int = nc.dram_tensor("out_int", [BL, I], dt.float32, kind="ExternalOutput")

    with tile.TileContext(nc) as tc:
        with (
            tc.tile_pool(name="wpool", bufs=1) as wp,
            tc.tile_pool(name="estage", bufs=2) as ep,
            tc.tile_pool(name="psPC", bufs=2, space="PSUM") as psPC,
        ):
            # ---------------- resident tiles ----------------
            w_wh = wp.tile([P, 4 * 2048], dt.bfloat16)
            w_pt = wp.tile([P, 2048], dt.bfloat16)
            w_enc = wp.tile([P, 4 * 2048], dt.bfloat16)
            w_s = wp.tile([P, 4 * S], dt.bfloat16)
            w_attw = wp.tile([P, 4], dt.bfloat16)
            t_biasg = wp.tile([P, 16], dt.float32)
            t_bslot = wp.tile([BL, S], dt.float32)
            t_lens = wp.tile([BL, 1], dt.float32)
            id8 = wp.tile([8, 8], dt.bfloat16)
            valid = wp.tile([BL, T], dt.float32)
            gconst = wp.tile([P, 16 * BL], dt.float32)
            genc = [wp.tile([P, BLK * 16 * BL], dt.float32, tag=f"genc{j}",
                            name=f"genc{j}") for j in range(2)]
            t_h = wp.tile([P, 4 * BL], dt.float32)
            t_c = wp.tile([P, 4 * BL], dt.float32)
            t_hbf = wp.tile([P, 4 * BL], dt.bfloat16)
            t_oh = wp.tile([P, BL], dt.bfloat16)
            vstage = wp.tile([BL, 2 * BLK], dt.float32)

            for hc in range(4):
                nc.sync.dma_start(w_wh[:, hc * 2048:(hc + 1) * 2048],
                                  whT[ds(hc * P, P), :])
                nc.sync.dma_start(w_enc[:, hc * 2048:(hc + 1) * 2048],
                                  wencT[ds(hc * P, P), :])
                nc.sync.dma_start(w_s[:, hc * S:(hc + 1) * S], wsT[ds(hc * P, P), :])
            nc.sync.dma_start(w_pt[:], pT[:])
            nc.sync.dma_start(w_attw[:], attw[:])
            nc.sync.dma_start(t_biasg[:], bias_g[:])
            nc.sync.dma_start(t_bslot[:], b_slot[:])
            nc.sync.dma_start(t_lens[:], lens[:])
            nc.sync.dma_start(t_h[:], h0[:])
            nc.sync.dma_start(t_c[:], c0[:])
            make_identity(nc, id8[:])
            nc.vector.tensor_copy(t_hbf[:], t_h[:])
            nc.vector.memset(t_oh[:], 0.0)

            # ---------------- setup (attention, const gates, intent) --------
            with (
                tc.tile_pool(name="setsb", bufs=2) as sp,
                tc.tile_pool(name="setps", bufs=2, space="PSUM") as psA,
            ):
                # valid mask + (valid-1)*1e9
                iot = sp.tile([BL, T], dt.int32)
                nc.gpsimd.iota(iot[:], [[1, T]], channel_multiplier=0)
                iotf = sp.tile([BL, T], dt.float32)
                nc.vector.tensor_copy(iotf[:], iot[:])
                nc.vector.tensor_scalar(valid[:], iotf[:], t_lens[:, 0:1], None,
                                        op0=ALU.is_lt)
                nmask = sp.tile([BL, T], dt.float32)
                nc.vector.tensor_scalar(nmask[:], valid[:], 1.0, 1e9,
                                        op0=ALU.subtract, op1=ALU.mult)

                # attention logits: [8, T]
                lgT = sp.tile([BL, T], dt.float32)
                for nb in range(16):
                    b_, th = nb // 2, (nb % 2) * 512
                    ps_at = psA.tile([1, 512], dt.float32, tag="su")
                    for hc in range(4):
                        sl = ep.tile([P, 512], dt.bfloat16, tag="att_in")
                        nc.sync.dma_start(sl[:], encT[ds(hc * P, P), b_, ds(th, 512)])
                        nc.tensor.matmul(ps_at[:], w_attw[:, hc:hc + 1], sl[:],
                                         start=(hc == 0), stop=(hc == 3))
                    lgst = sp.tile([1, 512], dt.float32, tag="lgst")
                    nc.vector.tensor_copy(lgst[:], ps_at[:])
                    nc.sync.dma_start(lgT[b_:b_ + 1, th:th + 512], lgst[:])

                # masked softmax over t + pooled-mean weights
                ml = sp.tile([BL, T], dt.float32)
                nc.vector.tensor_tensor(ml[:], lgT[:], nmask[:], op=ALU.add)
                rmx = sp.tile([BL, 1], dt.float32)
                nc.vector.reduce_max(rmx[:], ml[:], axis=AXX)
                nmx = sp.tile([BL, 1], dt.float32)
                nc.vector.tensor_scalar_mul(nmx[:], rmx[:], -1.0)
                ex = sp.tile([BL, T], dt.float32)
                sme = sp.tile([BL, 1], dt.float32)
                nc.scalar.activation(ex[:], ml[:], AF.Exp, bias=nmx[:],
                                     accum_out=sme[:])
                rcp = sp.tile([BL, 1], dt.float32)
                nc.vector.reciprocal(rcp[:], sme[:])
                aw = sp.tile([BL, T], dt.bfloat16)
                nc.vector.tensor_scalar(aw[:], ex[:], rcp[:, 0:1], None, op0=ALU.mult)
                rlen = sp.tile([BL, 1], dt.float32)
                nc.vector.reciprocal(rlen[:], t_lens[:])
                pw = sp.tile([BL, T], dt.bfloat16)
                nc.vector.tensor_scalar(pw[:], valid[:], rlen[:, 0:1], None,
                                        op0=ALU.mult)

                # transposed weights awp[128, (tc, b, {attn,pool})]
                awp = sp.tile([P, 8 * BL * 2], dt.bfloat16)
                for tc_ in range(8):
                    for k, src in enumerate((aw, pw)):
                        pst = psA.tile([P, 8], dt.bfloat16, tag="tp")
                        nc.tensor.transpose(pst[:], src[:, tc_ * P:(tc_ + 1) * P],
                                            id8[:])
                        nc.vector.tensor_copy(
                            awp[:].rearrange("p (t b k) -> p t b k", t=8, b=BL,
                                             k=2)[:, tc_, :, k], pst[:])

                # context / pooled
                ctxb = sp.tile([BL, H], dt.bfloat16)
                poolb = sp.tile([BL, H], dt.bfloat16)
                for b_ in range(BL):
                    ps_cp = psA.tile([2, H], dt.float32, tag="su")
                    for tc_ in range(8):
                        en = ep.tile([P, H], dt.bfloat16, tag="cp_in")
                        nc.sync.dma_start(en[:], encN[b_, ds(tc_ * P, P), :])
                        nc.tensor.matmul(
                            ps_cp[:],
                            awp[:, tc_ * 16 + 2 * b_: tc_ * 16 + 2 * b_ + 2],
                            en[:], start=(tc_ == 0), stop=(tc_ == 7))
                    cpst = sp.tile([2, H], dt.bfloat16, tag="cpst")
                    nc.vector.tensor_copy(cpst[:], ps_cp[:])
                    nc.sync.dma_start(ctxb[b_:b_ + 1, :], cpst[0:1, :])
                    nc.sync.dma_start(poolb[b_:b_ + 1, :], cpst[1:2, :])

                # xT[128, (xc, b)]: xc 0-3 = context chunks, 4-7 = pooled
                xT = sp.tile([P, 8 * BL], dt.bfloat16)
                for xc in range(8):
                    src = ctxb if xc < 4 else poolb
                    pst2 = psA.tile([P, 8], dt.bfloat16, tag="tp")
                    nc.tensor.transpose(pst2[:], src[:, (xc % 4) * P:(xc % 4 + 1) * P],
                                        id8[:])
                    nc.vector.tensor_copy(xT[:, xc * BL:(xc + 1) * BL], pst2[:])

                # g_const = W_ctx ctx + b_ih + b_hh   [128, (m, b)]
                w_ctx = sp.tile([P, 4 * 2048], dt.bfloat16)
                for hc in range(4):
                    nc.sync.dma_start(w_ctx[:, hc * 2048:(hc + 1) * 2048],
                                      wctxT[ds(hc * P, P), :])
                for m in range(16):
                    ps_gc = psA.tile([P, BL], dt.float32, tag="su")
                    for cc in range(4):
                        nc.tensor.matmul(
                            ps_gc[:],
                            w_ctx[:, cc * 2048 + m * P: cc * 2048 + (m + 1) * P],
                            xT[:, cc * BL:(cc + 1) * BL],
                            start=(cc == 0), stop=(cc == 3))
                    nc.vector.tensor_tensor(
                        gconst[:, m * BL:(m + 1) * BL], ps_gc[:],
                        t_biasg[:, m:m + 1].to_broadcast([P, BL]), op=ALU.add)

                # intent
                w_int = sp.tile([P, 8 * I], dt.bfloat16)
                t_bint = sp.tile([BL, I], dt.float32)
                nc.sync.dma_start(t_bint[:], b_int[:])
                for xc in range(8):
                    nc.sync.dma_start(w_int[:, xc * I:(xc + 1) * I],
                                      wintT[ds(xc * P, P), :])
                ps_in = psA.tile([BL, I], dt.float32, tag="su")
                for xc in range(8):
                    nc.tensor.matmul(ps_in[:], xT[:, xc * BL:(xc + 1) * BL],
                                     w_int[:, xc * I:(xc + 1) * I],
                                     start=(xc == 0), stop=(xc == 7))
                il = sp.tile([BL, I], dt.float32)
                nc.vector.tensor_tensor(
                    il[:], ps_in[:],
                    t_bint[:], op=ALU.add)
                imx = sp.tile([BL, 1], dt.float32)
                nc.vector.reduce_max(imx[:], il[:], axis=AXX)
                inx = sp.tile([BL, 1], dt.float32)
                nc.vector.tensor_scalar_mul(inx[:], imx[:], -1.0)
                iex = sp.tile([BL, I], dt.float32)
                ism = sp.tile([BL, 1], dt.float32)
                nc.scalar.activation(iex[:], il[:], AF.Exp, bias=inx[:],
                                     accum_out=ism[:])
                iln = sp.tile([BL, 1], dt.float32)
                nc.scalar.activation(iln[:], ism[:], AF.Ln)
                iof = sp.tile([BL, 1], dt.float32)
                nc.vector.tensor_tensor(iof[:], iln[:], imx[:], op=ALU.add)
                iout = sp.tile([BL, I], dt.float32)
                nc.vector.tensor_scalar(iout[:], il[:], iof[:, 0:1], None,
                                        op0=ALU.subtract)
                nc.sync.dma_start(out_int[:], iout[:])

                # g_emb0 = W_emb @ init_slot, pre-added into genc[0] step 0
                w_emb = sp.tile([P, 2048], dt.bfloat16)
                t_init = sp.tile([P, BL], dt.bfloat16)
                nc.sync.dma_start(w_emb[:], wembT[:])
                nc.sync.dma_start(t_init[:], initT8[:])
                ge0 = sp.tile([P, 16 * BL], dt.float32)
                for m in range(16):
                    ps_ge = psA.tile([P, BL], dt.float32, tag="su")
                    nc.tensor.matmul(ps_ge[:], w_emb[:, m * P:(m + 1) * P], t_init[:])
                    nc.scalar.copy(ge0[:, m * BL:(m + 1) * BL], ps_ge[:])

                def precompute(tb, buf, extra=None):
                    """genc[buf][:, t*128+m*8+b] = W_enc enc[:, tb+t] + g_const."""
                    slab = ep.tile([P, 4 * BL * BLK], dt.bfloat16, tag="slab")
                    for hc in range(4):
                        nc.sync.dma_start(
                            slab[:, hc * BL * BLK:(hc + 1) * BL * BLK]
                            .rearrange("p (b t) -> p b t", b=BL),
                            encT[ds(hc * P, P), :, ds(tb, BLK)])
                    gv = genc[buf][:].rearrange("p (t m b) -> p t m b",
                                                t=BLK, m=16, b=BL)
                    for m in range(16):
                        ps_pc = psPC.tile([P, BL * BLK], dt.float32, tag="ps_pc")
                        for hc in range(4):
                            nc.tensor.matmul(
                                ps_pc[:],
                                w_enc[:, hc * 2048 + m * P: hc * 2048 + (m + 1) * P],
                                slab[:, hc * BL * BLK:(hc + 1) * BL * BLK],
                                start=(hc == 0), stop=(hc == 3))
                        nc.vector.tensor_tensor(
                            gv[:, :, m, :],
                            ps_pc[:].rearrange("p (b t) -> p t b", b=BL),
                            gconst[:, m * BL:(m + 1) * BL].unsqueeze(1)
                            .broadcast_to((P, BLK, BL)),
                            op=ALU.add)
                    if extra is not None:
                        nc.vector.tensor_tensor(
                            genc[buf][:, 0:16 * BL], genc[buf][:, 0:16 * BL],
                            extra[:], op=ALU.add)

                precompute(0, 0, extra=ge0)
                precompute(BLK, 1)

            # ---------------- scan loop ----------------
            with (
                tc.tile_pool(name="scansb", bufs=2) as ss,
                tc.tile_pool(name="psG", bufs=2, space="PSUM") as psG,
                tc.tile_pool(name="psS", bufs=2, space="PSUM") as psS,
                tc.tile_pool(name="psO", bufs=2, space="PSUM") as psO,
            ):
                bslot_b = t_bslot[:]

                def step(t0, tbtl, buf, tl):
                    # gate matmuls: W_h h + P'' onehot  (accumulate per m-slice)
                    gps = psG.tile([P, 16 * BL], dt.float32, tag="gps")
                    for m in range(16):
                        o = m * BL
                        for hc in range(4):
                            nc.tensor.matmul(
                                gps[:, o:o + BL],
                                w_wh[:, hc * 2048 + m * P: hc * 2048 + (m + 1) * P],
                                t_hbf[:, hc * BL:(hc + 1) * BL],
                                start=(hc == 0), stop=False, skip_group_check=True)
                        nc.tensor.matmul(gps[:, o:o + BL], w_pt[:, m * P:(m + 1) * P],
                                         t_oh[:], start=False, stop=True,
                                         skip_group_check=True)
                    gsb = ss.tile([P, 16 * BL], dt.float32, tag="gsb")
                    nc.vector.tensor_tensor(gsb[:], gps[:],
                                            genc[buf][:, tl * 128:(tl + 1) * 128],
                                            op=ALU.add)
                    # LSTM cell (i: cols 0:32, f: 32:64, g: 64:96, o: 96:128)
                    act = ss.tile([P, 16 * BL], dt.float32, tag="act")
                    nc.scalar.activation(act[:, 0:8 * BL], gsb[:, 0:8 * BL], AF.Sigmoid)
                    nc.scalar.activation(act[:, 8 * BL:12 * BL], gsb[:, 8 * BL:12 * BL],
                                         AF.Tanh)
                    nc.scalar.activation(act[:, 12 * BL:16 * BL],
                                         gsb[:, 12 * BL:16 * BL], AF.Sigmoid)
                    fc = ss.tile([P, 4 * BL], dt.float32, tag="fc")
                    nc.vector.tensor_tensor(fc[:], act[:, 4 * BL:8 * BL], t_c[:],
                                            op=ALU.mult)
                    ig = ss.tile([P, 4 * BL], dt.float32, tag="ig")
                    nc.vector.tensor_tensor(ig[:], act[:, 0:4 * BL],
                                            act[:, 8 * BL:12 * BL], op=ALU.mult)
                    nc.vector.tensor_tensor(t_c[:], fc[:], ig[:], op=ALU.add)
                    tch = ss.tile([P, 4 * BL], dt.float32, tag="tch")
                    nc.scalar.activation(tch[:], t_c[:], AF.Tanh)
                    nc.vector.tensor_tensor(t_h[:], act[:, 12 * BL:16 * BL], tch[:],
                                            op=ALU.mult)
                    nc.vector.tensor_copy(t_hbf[:], t_h[:])
                    # slot head
                    sps = psS.tile([BL, S], dt.float32, tag="sps")
                    for hc in range(4):
                        nc.tensor.matmul(sps[:], t_hbf[:, hc * BL:(hc + 1) * BL],
                                         w_s[:, hc * S:(hc + 1) * S],
                                         start=(hc == 0), stop=(hc == 3))
                    slb = ss.tile([BL, S], dt.float32, tag="slb")
                    nc.vector.tensor_tensor(slb[:], sps[:], bslot_b, op=ALU.add)
                    smx = ss.tile([BL, 1], dt.float32, tag="smx")
                    nc.vector.reduce_max(smx[:], slb[:], axis=AXX)
                    snx = ss.tile([BL, 1], dt.float32, tag="snx")
                    nc.vector.tensor_scalar_mul(snx[:], smx[:], -1.0)
                    sex = ss.tile([BL, S], dt.float32, tag="sex")
                    ssm = ss.tile([BL, 1], dt.float32, tag="ssm")
                    nc.scalar.activation(sex[:], slb[:], AF.Exp, bias=snx[:],
                                         accum_out=ssm[:])
                    sln = ss.tile([BL, 1], dt.float32, tag="sln")
                    nc.scalar.activation(sln[:], ssm[:], AF.Ln)
                    sof = ss.tile([BL, 1], dt.float32, tag="sof")
                    nc.vector.tensor_tensor(sof[:], sln[:], smx[:], op=ALU.add)
                    lp = ss.tile([BL, S], dt.float32, tag="lp")
                    nc.vector.tensor_scalar(lp[:], slb[:], sof[:, 0:1],
                                            vstage[:, tbtl:tbtl + 1],
                                            op0=ALU.subtract, op1=ALU.mult)
                    nc.sync.dma_start(out_lp[:, ds(t0 + tbtl, 1), :],
                                      lp[:].unsqueeze(1))
                    # onehot for next step's emb matmul
                    ohA = ss.tile([BL, S], dt.bfloat16, tag="ohA")
                    nc.vector.tensor_scalar(ohA[:], slb[:], smx[:, 0:1], None,
                                            op0=ALU.is_equal)
                    pso = psO.tile([P, BL], dt.bfloat16, tag="pso")
                    nc.tensor.transpose(pso[:], ohA[:], id8[:])
                    nc.vector.tensor_copy(t_oh[:], pso[:])

                with tc.For_i(0, T, 2 * BLK) as t0:
                    nc.sync.dma_start(vstage[:], valid[:, ds(t0, 2 * BLK)])
                    for sub in range(2):
                        for tl in range(BLK):
                            step(t0, sub * BLK + tl, sub, tl)
                        precompute(t0 + 2 * BLK + sub * BLK, sub)
    nc.finalize()
    return nc


def _prep(inputs):
    f32 = np.float32
    enc = np.asarray(inputs["encoder_hiddens"], f32)
    W_ih = np.asarray(inputs["W_ih"], f32)
    W_hh = np.asarray(inputs["W_hh"], f32)
    W_h = (W_ih[:, :H] + W_hh)
    W_emb = W_ih[:, H:H + E]
    W_enc = W_ih[:, H + E:H + E + H]
    W_ctx = W_ih[:, H + E + H:]
    table = np.asarray(inputs["slot_emb_table"], f32)
    pT = (table @ W_emb.T)
    lens = np.asarray(inputs["seq_lens"]).astype(np.int64)
    att_we = np.asarray(inputs["att_w"], f32)[H:, 0]
    biasg = (np.asarray(inputs["b_ih"], f32) + np.asarray(inputs["b_hh"], f32))
    bf = lambda x: np.ascontiguousarray(x, dtype=bf16)
    shared = {
        "whT": bf(W_h.T), "pT": bf(pT), "wencT": bf(W_enc.T),
        "wctxT": bf(W_ctx.T), "wsT": bf(np.asarray(inputs["W_slot"], f32).T),
        "wintT": bf(np.asarray(inputs["W_int"], f32).T),
        "wembT": bf(W_emb.T),
        "initT8": bf(np.repeat(np.asarray(inputs["init_slot"], f32).T, BL, 1)),
        "attw": bf(att_we.reshape(4, P).T),
        "bias_g": np.ascontiguousarray(biasg.reshape(16, P).T, dtype=f32),
        "b_slot": np.broadcast_to(np.asarray(inputs["b_slot"], f32)[None, :],
                                  (BL, S)).copy(),
        "b_int": np.broadcast_to(np.asarray(inputs["b_int"], f32)[None, :],
                                 (BL, I)).copy(),
    }
    h_all = np.asarray(inputs["lstm_h"], f32)
    c_all = np.asarray(inputs["lstm_c"], f32)
    in_maps = []
    for k in range(NC):
        sl = slice(k * BL, (k + 1) * BL)
        encl = enc[sl]                                   # [8, T, H]
        encTl = np.zeros((H, BL, TPAD), dtype=bf16)
        encTl[:, :, :T] = encl.transpose(2, 0, 1).astype(bf16)
        m = dict(shared)
        m["encT"] = encTl
        m["encN"] = bf(encl)
        m["lens"] = lens[sl].astype(f32)[:, None]
        m["h0"] = np.ascontiguousarray(
            h_all[sl].T.reshape(4, P, BL).transpose(1, 0, 2).reshape(P, 4 * BL),
            dtype=f32)
        m["c0"] = np.ascontiguousarray(
            c_all[sl].T.reshape(4, P, BL).transpose(1, 0, 2).reshape(P, 4 * BL),
            dtype=f32)
        in_maps.append(m)
    return in_maps


def kernel(**inputs):
    if "nc" not in _cache:
        _cache["nc"] = build_kernel()
    in_maps = _prep(inputs)
    res = run_bass_kernel_spmd(_cache["nc"], in_maps, core_ids=list(range(NC)))
    slot_logp = np.concatenate([r["out_lp"] for r in res.results], axis=0)
    intent = np.concatenate([r["out_int"] for r in res.results], axis=0)
    return slot_logp, intent


# revision 17
# speedup vs baseline: 1.0033x; 1.0033x over previous
"""Trainium2 Bass kernel for nn_Decoder (LSTM slot decoder, ragged sequences).

Sharding: data-parallel over batch across 8 NeuronCores (8 examples/core),
small weights replicated (spec sharding_hint). Per core the LSTM scan runs
sequentially over T=1024 steps with gates held in a [128-partition
gate-chunk, batch] layout so the elementwise cell runs at full vector
width. Matmuls are bf16 (measured ~1.7e-3 end-to-end rel err vs the fp32
reference; the LSTM contracts argmax-flip perturbations); state, PSUM
accumulation and activations stay fp32.

Per-step (B=8 local, H=512, S=128, 4H=2048 gates):
  gates = W_h h + P'' onehot_prev + g_enc[t] + g_const     (PSUM accum)
  c = sig(f) c + sig(i) tanh(g);  h = sig(o) tanh(c)
  slot = h W_slot^T (+b);  logp = log_softmax(slot) * valid[t] -> HBM
  onehot = (slot == rowmax)    # emb lookup becomes a matmul with P''
with W_h = W_ih[:, :H] + W_hh (both multiply the carry h) and
P'' = slot_emb_table @ W_emb^T. g_enc[t] = W_enc enc_t + g_const is
precomputed in 32-step double-buffered blocks interleaved with the scan.
The state freeze past seq_len in the reference is unnecessary: frozen
steps only produce outputs that are masked to zero, so we skip it.
"""

import numpy as np
import ml_dtypes

import concourse.bass as bass
import concourse.mybir as mybir
import concourse.tile as tile
from concourse import bacc
from concourse.bass import ds
from concourse.bass_utils import run_bass_kernel_spmd
from concourse.masks import make_identity

dt = mybir.dt
AF = mybir.ActivationFunctionType
ALU = mybir.AluOpType
AXX = mybir.AxisListType.X
P = 128
B, T, H, E, S, I = 64, 1024, 512, 128, 128, 32
NC = 8
BL = B // NC          # local batch
BLK = 32              # precompute block size (steps)
TPAD = T + 2 * BLK
bf16 = ml_dtypes.bfloat16

_cache = {}


def build_kernel(passes=1):
    nc = bacc.Bacc(None, target_bir_lowering=False)
    di = lambda n, sh, d=dt.bfloat16: nc.dram_tensor(n, sh, d, kind="ExternalInput")
    encT = di("encT", [H, BL, TPAD])
    encN = di("encN", [BL, T, H])
    whT = di("whT", [H, 4 * H])
    pT = di("pT", [S, 4 * H])
    wencT = di("wencT", [H, 4 * H])
    wctxT = di("wctxT", [H, 4 * H])
    wsT = di("wsT", [H, S])
    wintT = di("wintT", [2 * H, I])
    wembT = di("wembT", [E, 4 * H])
    initT8 = di("initT8", [E, BL])
    attw = di("attw", [P, 4])
    bias_g = di("bias_g", [P, 16], dt.float32)
    b_slot = di("b_slot", [BL, S], dt.float32)
    b_int = di("b_int", [BL, I], dt.float32)
    lens = di("lens", [BL, 1], dt.float32)
    h0 = di("h0", [P, 4 * BL])  # bf16
    c0 = di("c0", [P, 4 * BL], dt.float32)
    out_lp = nc.dram_tensor("out_lp", [BL, T, S], dt.float32, kind="ExternalOutput")
    out_int = nc.dram_tensor("out_int", [BL, I], dt.float32, kind="ExternalOutput")

    with tile.TileContext(nc) as tc:
        with (
            tc.tile_pool(name="wpool", bufs=1) as wp,
            tc.tile_pool(name="estage", bufs=2) as ep,
            tc.tile_pool(name="psPC", bufs=2, space="PSUM") as psPC,
        ):
            # ---------------- resident tiles ----------------
            w_wh = wp.tile([P, 4 * 2048], dt.bfloat16)
            w_pt = wp.tile([P, 2048], dt.bfloat16)
            w_enc = wp.tile([P, 4 * 2048], dt.bfloat16)
            w_s = wp.tile([P, 4 * S], dt.bfloat16)
            w_attw = wp.tile([P, 4], dt.bfloat16)
            t_biasg = wp.tile([P, 16], dt.float32)
            t_bslot = wp.tile([BL, S], dt.float32)
            t_lens = wp.tile([BL, 1], dt.float32)
            id8 = wp.tile([8, 8], dt.bfloat16)
            valid = wp.tile([BL, T], dt.float32)
            gconst = wp.tile([P, 16 * BL], dt.float32)
            genc = [wp.tile([P, BLK * 16 * BL], dt.float32, tag=f"genc{j}",
                            name=f"genc{j}") for j in range(2)]
            t_c = wp.tile([P, 4 * BL], dt.float32)
            t_hbf = wp.tile([P, 4 * BL], dt.bfloat16)
            t_oh = wp.tile([P, BL], dt.bfloat16)
            t_ohA = wp.tile([BL, S], dt.bfloat16)
            vstage = wp.tile([BL, 2 * BLK], dt.float32)

            for hc in range(4):
                nc.sync.dma_start(w_wh[:, hc * 2048:(hc + 1) * 2048],
                                  whT[ds(hc * P, P), :])
                nc.sync.dma_start(w_enc[:, hc * 2048:(hc + 1) * 2048],
                                  wencT[ds(hc * P, P), :])
                nc.sync.dma_start(w_s[:, hc * S:(hc + 1) * S], wsT[ds(hc * P, P), :])
            nc.sync.dma_start(w_pt[:], pT[:])
            nc.sync.dma_start(w_attw[:], attw[:])
            nc.sync.dma_start(t_biasg[:], bias_g[:])
            nc.sync.dma_start(t_bslot[:], b_slot[:])
            nc.sync.dma_start(t_lens[:], lens[:])
            nc.sync.dma_start(t_c[:], c0[:])
            make_identity(nc, id8[:])
            nc.sync.dma_start(t_hbf[:], h0[:])
            nc.vector.memset(t_ohA[:], 0.0)

            # ---------------- setup (attention, const gates, intent) --------
            with (
                tc.tile_pool(name="setsb", bufs=2) as sp,
                tc.tile_pool(name="setps", bufs=2, space="PSUM") as psA,
            ):
                # valid mask + (valid-1)*1e9
                iot = sp.tile([BL, T], dt.int32)
                nc.gpsimd.iota(iot[:], [[1, T]], channel_multiplier=0)
                iotf = sp.tile([BL, T], dt.float32)
                nc.vector.tensor_copy(iotf[:], iot[:])
                nc.vector.tensor_scalar(valid[:], iotf[:], t_lens[:, 0:1], None,
                                        op0=ALU.is_lt)
                nmask = sp.tile([BL, T], dt.float32)
                nc.vector.tensor_scalar(nmask[:], valid[:], 1.0, 1e9,
                                        op0=ALU.subtract, op1=ALU.mult)

                # attention logits: [8, T]
                lgT = sp.tile([BL, T], dt.float32)
                for nb in range(16):
                    b_, th = nb // 2, (nb % 2) * 512
                    ps_at = psA.tile([1, 512], dt.float32, tag="su")
                    for hc in range(4):
                        sl = ep.tile([P, 512], dt.bfloat16, tag="att_in")
                        nc.sync.dma_start(sl[:], encT[ds(hc * P, P), b_, ds(th, 512)])
                        nc.tensor.matmul(ps_at[:], w_attw[:, hc:hc + 1], sl[:],
                                         start=(hc == 0), stop=(hc == 3))
                    lgst = sp.tile([1, 512], dt.float32, tag="lgst")
                    nc.vector.tensor_copy(lgst[:], ps_at[:])
                    nc.sync.dma_start(lgT[b_:b_ + 1, th:th + 512], lgst[:])

                # masked softmax over t + pooled-mean weights
                ml = sp.tile([BL, T], dt.float32)
                nc.vector.tensor_tensor(ml[:], lgT[:], nmask[:], op=ALU.add)
                rmx = sp.tile([BL, 1], dt.float32)
                nc.vector.reduce_max(rmx[:], ml[:], axis=AXX)
                nmx = sp.tile([BL, 1], dt.float32)
                nc.vector.tensor_scalar_mul(nmx[:], rmx[:], -1.0)
                ex = sp.tile([BL, T], dt.float32)
                sme = sp.tile([BL, 1], dt.float32)
                nc.scalar.activation(ex[:], ml[:], AF.Exp, bias=nmx[:],
                                     accum_out=sme[:])
                rcp = sp.tile([BL, 1], dt.float32)
                nc.vector.reciprocal(rcp[:], sme[:])
                aw = sp.tile([BL, T], dt.bfloat16)
                nc.vector.tensor_scalar(aw[:], ex[:], rcp[:, 0:1], None, op0=ALU.mult)
                rlen = sp.tile([BL, 1], dt.float32)
                nc.vector.reciprocal(rlen[:], t_lens[:])
                pw = sp.tile([BL, T], dt.bfloat16)
                nc.vector.tensor_scalar(pw[:], valid[:], rlen[:, 0:1], None,
                                        op0=ALU.mult)

                # transposed weights awp[128, (tc, b, {attn,pool})]
                awp = sp.tile([P, 8 * BL * 2], dt.bfloat16)
                for tc_ in range(8):
                    for k, src in enumerate((aw, pw)):
                        pst = psA.tile([P, 8], dt.bfloat16, tag="tp")
                        nc.tensor.transpose(pst[:], src[:, tc_ * P:(tc_ + 1) * P],
                                            id8[:])
                        nc.vector.tensor_copy(
                            awp[:].rearrange("p (t b k) -> p t b k", t=8, b=BL,
                                             k=2)[:, tc_, :, k], pst[:])

                # context / pooled
                ctxb = sp.tile([BL, H], dt.bfloat16)
                poolb = sp.tile([BL, H], dt.bfloat16)
                for b_ in range(BL):
                    ps_cp = psA.tile([2, H], dt.float32, tag="su")
                    for tc_ in range(8):
                        en = ep.tile([P, H], dt.bfloat16, tag="cp_in")
                        nc.sync.dma_start(en[:], encN[b_, ds(tc_ * P, P), :])
                        nc.tensor.matmul(
                            ps_cp[:],
                            awp[:, tc_ * 16 + 2 * b_: tc_ * 16 + 2 * b_ + 2],
                            en[:], start=(tc_ == 0), stop=(tc_ == 7))
                    cpst = sp.tile([2, H], dt.bfloat16, tag="cpst")
                    nc.vector.tensor_copy(cpst[:], ps_cp[:])
                    nc.sync.dma_start(ctxb[b_:b_ + 1, :], cpst[0:1, :])
                    nc.sync.dma_start(poolb[b_:b_ + 1, :], cpst[1:2, :])

                # xT[128, (xc, b)]: xc 0-3 = context chunks, 4-7 = pooled
                xT = sp.tile([P, 8 * BL], dt.bfloat16)
                for xc in range(8):
                    src = ctxb if xc < 4 else poolb
                    pst2 = psA.tile([P, 8], dt.bfloat16, tag="tp")
                    nc.tensor.transpose(pst2[:], src[:, (xc % 4) * P:(xc % 4 + 1) * P],
                                        id8[:])
                    nc.vector.tensor_copy(xT[:, xc * BL:(xc + 1) * BL], pst2[:])

                # g_const = W_ctx ctx + b_ih + b_hh   [128, (m, b)]
                w_ctx = sp.tile([P, 4 * 2048], dt.bfloat16)
                for hc in range(4):
                    nc.sync.dma_start(w_ctx[:, hc * 2048:(hc + 1) * 2048],
                                      wctxT[ds(hc * P, P), :])
                for m in range(16):
                    ps_gc = psA.tile([P, BL], dt.float32, tag="su")
                    for cc in range(4):
                        nc.tensor.matmul(
                            ps_gc[:],
                            w_ctx[:, cc * 2048 + m * P: cc * 2048 + (m + 1) * P],
                            xT[:, cc * BL:(cc + 1) * BL],
                            start=(cc == 0), stop=(cc == 3))
                    nc.vector.tensor_tensor(
                        gconst[:, m * BL:(m + 1) * BL], ps_gc[:],
                        t_biasg[:, m:m + 1].to_broadcast([P, BL]), op=ALU.add)

                # intent
                w_int = sp.tile([P, 8 * I], dt.bfloat16)
                t_bint = sp.tile([BL, I], dt.float32)
                nc.sync.dma_start(t_bint[:], b_int[:])
                for xc in range(8):
                    nc.sync.dma_start(w_int[:, xc * I:(xc + 1) * I],
                                      wintT[ds(xc * P, P), :])
                ps_in = psA.tile([BL, I], dt.float32, tag="su")
                for xc in range(8):
                    nc.tensor.matmul(ps_in[:], xT[:, xc * BL:(xc + 1) * BL],
                                     w_int[:, xc * I:(xc + 1) * I],
                                     start=(xc == 0), stop=(xc == 7))
                il = sp.tile([BL, I], dt.float32)
                nc.vector.tensor_tensor(
                    il[:], ps_in[:],
                    t_bint[:], op=ALU.add)
                imx = sp.tile([BL, 1], dt.float32)
                nc.vector.reduce_max(imx[:], il[:], axis=AXX)
                inx = sp.tile([BL, 1], dt.float32)
                nc.vector.tensor_scalar_mul(inx[:], imx[:], -1.0)
                iex = sp.tile([BL, I], dt.float32)
                ism = sp.tile([BL, 1], dt.float32)
                nc.scalar.activation(iex[:], il[:], AF.Exp, bias=inx[:],
                                     accum_out=ism[:])
                iln = sp.tile([BL, 1], dt.float32)
                nc.scalar.activation(iln[:], ism[:], AF.Ln)
                iof = sp.tile([BL, 1], dt.float32)
                nc.vector.tensor_tensor(iof[:], iln[:], imx[:], op=ALU.add)
                iout = sp.tile([BL, I], dt.float32)
                nc.vector.tensor_scalar(iout[:], il[:], iof[:, 0:1], None,
                                        op0=ALU.subtract)
                nc.sync.dma_start(out_int[:], iout[:])

                # g_emb0 = W_emb @ init_slot, pre-added into genc[0] step 0
                w_emb = sp.tile([P, 2048], dt.bfloat16)
                t_init = sp.tile([P, BL], dt.bfloat16)
                nc.sync.dma_start(w_emb[:], wembT[:])
                nc.sync.dma_start(t_init[:], initT8[:])
                ge0 = sp.tile([P, 16 * BL], dt.float32)
                for m in range(16):
                    ps_ge = psA.tile([P, BL], dt.float32, tag="su")
                    nc.tensor.matmul(ps_ge[:], w_emb[:, m * P:(m + 1) * P], t_init[:])
                    nc.scalar.copy(ge0[:, m * BL:(m + 1) * BL], ps_ge[:])

                def precompute(tb, buf, extra=None):
                    """genc[buf][:, t*128+m*8+b] = W_enc enc[:, tb+t] + g_const."""
                    slab = ep.tile([P, 4 * BL * BLK], dt.bfloat16, tag="slab")
                    for hc in range(4):
                        nc.sync.dma_start(
                            slab[:, hc * BL * BLK:(hc + 1) * BL * BLK]
                            .rearrange("p (b t) -> p b t", b=BL),
                            encT[ds(hc * P, P), :, ds(tb, BLK)])
                    gv = genc[buf][:].rearrange("p (t m b) -> p t m b",
                                                t=BLK, m=16, b=BL)
                    for m in range(16):
                        ps_pc = psPC.tile([P, BL * BLK], dt.float32, tag="ps_pc")
                        for hc in range(4):
                            nc.tensor.matmul(
                                ps_pc[:],
                                w_enc[:, hc * 2048 + m * P: hc * 2048 + (m + 1) * P],
                                slab[:, hc * BL * BLK:(hc + 1) * BL * BLK],
                                start=(hc == 0), stop=(hc == 3))
                        nc.vector.tensor_tensor(
                            gv[:, :, m, :],
                            ps_pc[:].rearrange("p (b t) -> p t b", b=BL),
                            gconst[:, m * BL:(m + 1) * BL].unsqueeze(1)
                            .broadcast_to((P, BLK, BL)),
                            op=ALU.add)
                    if extra is not None:
                        nc.vector.tensor_tensor(
                            genc[buf][:, 0:16 * BL], genc[buf][:, 0:16 * BL],
                            extra[:], op=ALU.add)

                precompute(0, 0, extra=ge0)
                precompute(BLK, 1)

            # ---------------- scan loop ----------------
            with (
                tc.tile_pool(name="scansb", bufs=2) as ss,
                tc.tile_pool(name="psG", bufs=2, space="PSUM") as psG,
                tc.tile_pool(name="psS", bufs=2, space="PSUM") as psS,
                tc.tile_pool(name="psO", bufs=2, space="PSUM") as psO,
            ):
                bslot_b = t_bslot[:]

                def step(t0, tbtl, buf, tl):
                    # A: W_h matmuls for gates(t) (rhs = h from prev step)
                    gps = psG.tile([P, 16 * BL], dt.float32, tag="gps")
                    for m in range(16):
                        o = m * BL
                        for hc in range(4):
                            nc.tensor.matmul(
                                gps[:, o:o + BL],
                                w_wh[:, hc * 2048 + m * P: hc * 2048 + (m + 1) * P],
                                t_hbf[:, hc * BL:(hc + 1) * BL],
                                start=(hc == 0), stop=False, skip_group_check=True)
                    # B: transpose prev step's onehot (zeros at t=0)
                    pso = psO.tile([P, BL], dt.bfloat16, tag="pso")
                    nc.tensor.transpose(pso[:], t_ohA[:], id8[:])
                    nc.scalar.copy(t_oh[:], pso[:])
                    # C: emb contribution via P'' @ onehot
                    for m in range(16):
                        nc.tensor.matmul(gps[:, m * BL:(m + 1) * BL],
                                         w_pt[:, m * P:(m + 1) * P],
                                         t_oh[:], start=False, stop=True,
                                         skip_group_check=True)
                    # D: LSTM cell (gate order i,f,o,g after host permute)
                    gsb = ss.tile([P, 16 * BL], dt.float32, tag="gsb")
                    nc.vector.tensor_tensor(gsb[:], gps[:],
                                            genc[buf][:, tl * 128:(tl + 1) * 128],
                                            op=ALU.add)
                    act = ss.tile([P, 16 * BL], dt.float32, tag="act")
                    nc.scalar.activation(act[:, 0:12 * BL], gsb[:, 0:12 * BL],
                                         AF.Sigmoid)
                    nc.scalar.activation(act[:, 12 * BL:16 * BL],
                                         gsb[:, 12 * BL:16 * BL], AF.Tanh)
                    fc = ss.tile([P, 4 * BL], dt.float32, tag="fc")
                    nc.vector.tensor_tensor(fc[:], act[:, 4 * BL:8 * BL], t_c[:],
                                            op=ALU.mult)
                    ig = ss.tile([P, 4 * BL], dt.float32, tag="ig")
                    nc.vector.tensor_tensor(ig[:], act[:, 0:4 * BL],
                                            act[:, 12 * BL:16 * BL], op=ALU.mult)
                    nc.vector.tensor_tensor(t_c[:], fc[:], ig[:], op=ALU.add)
                    tch = ss.tile([P, 4 * BL], dt.float32, tag="tch")
                    nc.scalar.activation(tch[:], t_c[:], AF.Tanh)
                    nc.vector.tensor_tensor(t_hbf[:], act[:, 8 * BL:12 * BL], tch[:],
                                            op=ALU.mult)
                    # E: slot head
                    sps = psS.tile([BL, S], dt.float32, tag="sps")
                    for hc in range(4):
                        nc.tensor.matmul(sps[:], t_hbf[:, hc * BL:(hc + 1) * BL],
                                         w_s[:, hc * S:(hc + 1) * S],
                                         start=(hc == 0), stop=(hc == 3))
                    # F: argmax onehot fast path, then softmax tail
                    slb = ss.tile([BL, S], dt.float32, tag="slb")
                    nc.vector.tensor_tensor(slb[:], sps[:], bslot_b, op=ALU.add)
                    smx = ss.tile([BL, 1], dt.float32, tag="smx")
                    nc.vector.reduce_max(smx[:], slb[:], axis=AXX)
                    nc.vector.tensor_scalar(t_ohA[:], slb[:], smx[:, 0:1], None,
                                            op0=ALU.is_equal)
                    snx = ss.tile([BL, 1], dt.float32, tag="snx")
                    nc.scalar.mul(snx[:], smx[:], -1.0)
                    sex = ss.tile([BL, S], dt.float32, tag="sex")
                    ssm = ss.tile([BL, 1], dt.float32, tag="ssm")
                    nc.scalar.activation(sex[:], slb[:], AF.Exp, bias=snx[:],
                                         accum_out=ssm[:])
                    sln = ss.tile([BL, 1], dt.float32, tag="sln")
                    nc.scalar.activation(sln[:], ssm[:], AF.Ln)
                    sof = ss.tile([BL, 1], dt.float32, tag="sof")
                    nc.scalar.add(sof[:], sln[:], smx[:, 0:1])
                    lp = ss.tile([BL, S], dt.float32, tag="lp")
                    nc.vector.tensor_scalar(lp[:], slb[:], sof[:, 0:1],
                                            vstage[:, tbtl:tbtl + 1],
                                            op0=ALU.subtract, op1=ALU.mult)
                    nc.sync.dma_start(out_lp[:, ds(t0 + tbtl, 1), :],
                                      lp[:].unsqueeze(1))

                HINTS = (mybir.EngineType.PE, mybir.EngineType.DVE,
                         mybir.EngineType.Activation, mybir.EngineType.Pool,
                         mybir.EngineType.SP)

                def scan_loop():
                    with tc.For_i(0, T, 2 * BLK, hint_engines=HINTS) as t0:
                        nc.sync.dma_start(vstage[:], valid[:, ds(t0, 2 * BLK)])
                        for sub in range(2):
                            for tl in range(BLK):
                                step(t0, sub * BLK + tl, sub, tl)
                            precompute(t0 + 2 * BLK + sub * BLK, sub)

                if passes == 1:
                    scan_loop()
                else:
                    with tc.For_i(0, passes, 1):
                        scan_loop()
    nc.finalize()
    return nc


def _prep(inputs):
    f32 = np.float32
    enc = np.asarray(inputs["encoder_hiddens"], f32)
    W_ih = np.asarray(inputs["W_ih"], f32)
    W_hh = np.asarray(inputs["W_hh"], f32)
    # permute gate rows (i,f,g,o) -> (i,f,o,g) so sigmoid is one slice
    perm = np.concatenate([np.arange(0, 2 * H), np.arange(3 * H, 4 * H),
                           np.arange(2 * H, 3 * H)])
    W_h = (W_ih[:, :H] + W_hh)[perm]
    W_emb = W_ih[perm, H:H + E]
    W_enc = W_ih[perm, H + E:H + E + H]
    W_ctx = W_ih[perm, H + E + H:]
    table = np.asarray(inputs["slot_emb_table"], f32)
    pT = (table @ W_emb.T)
    lens = np.asarray(inputs["seq_lens"]).astype(np.int64)
    att_we = np.asarray(inputs["att_w"], f32)[H:, 0]
    biasg = (np.asarray(inputs["b_ih"], f32) + np.asarray(inputs["b_hh"], f32))[perm]
    bf = lambda x: np.ascontiguousarray(x, dtype=bf16)
    shared = {
        "whT": bf(W_h.T), "pT": bf(pT), "wencT": bf(W_enc.T),
        "wctxT": bf(W_ctx.T), "wsT": bf(np.asarray(inputs["W_slot"], f32).T),
        "wintT": bf(np.asarray(inputs["W_int"], f32).T),
        "wembT": bf(W_emb.T),
        "initT8": bf(np.repeat(np.asarray(inputs["init_slot"], f32).T, BL, 1)),
        "attw": bf(att_we.reshape(4, P).T),
        "bias_g": np.ascontiguousarray(biasg.reshape(16, P).T, dtype=f32),
        "b_slot": np.broadcast_to(np.asarray(inputs["b_slot"], f32)[None, :],
                                  (BL, S)).copy(),
        "b_int": np.broadcast_to(np.asarray(inputs["b_int"], f32)[None, :],
                                 (BL, I)).copy(),
    }
    h_all = np.asarray(inputs["lstm_h"], f32)
    c_all = np.asarray(inputs["lstm_c"], f32)
    in_maps = []
    for k in range(NC):
        sl = slice(k * BL, (k + 1) * BL)
        encl = enc[sl]                                   # [8, T, H]
        encTl = np.zeros((H, BL, TPAD), dtype=bf16)
        encTl[:, :, :T] = encl.transpose(2, 0, 1).astype(bf16)
        m = dict(shared)
        m["encT"] = encTl
        m["encN"] = bf(encl)
        m["lens"] = lens[sl].astype(f32)[:, None]
        m["h0"] = np.ascontiguousarray(
            h_all[sl].T.reshape(4, P, BL).transpose(1, 0, 2).reshape(P, 4 * BL),
            dtype=bf16)
        m["c0"] = np.ascontiguousarray(
            c_all[sl].T.reshape(4, P, BL).transpose(1, 0, 2).reshape(P, 4 * BL),
            dtype=f32)
        in_maps.append(m)
    return in_maps


def kernel(**inputs):
    if "nc" not in _cache:
        _cache["nc"] = build_kernel()
    in_maps = _prep(inputs)
    res = run_bass_kernel_spmd(_cache["nc"], in_maps, core_ids=list(range(NC)))
    slot_logp = np.concatenate([r["out_lp"] for r in res.results], axis=0)
    intent = np.concatenate([r["out_int"] for r in res.results], axis=0)
    return slot_logp, intent
